# revision 27
# baseline (speedup 1.0000x reference)
"""Trainium2 Bass kernel for nn_AttentionBlock (GroupNorm + 1-head self-attention).

Reference computation (per batch b, C=256 channels, N=4096 spatial):
    xn = GroupNorm(x; 32 groups, eps=1e-6) * gn_w + gn_b
    q/k/v = W @ xn + b          (1x1 conv == channel matmul)
    attn  = softmax(q^T k / 16, axis=j)
    out   = x + Wo @ (v @ attn^T) + bo

Sharding: 8 cores = 4 batches x 2 query-halves. Each core computes
GroupNorm + K/V for its whole batch (duplicated across the pair) and
attention + output projection for its 2048 query rows.

Per-core x is sent with its own query columns rotated to the front
(attention is permutation-equivariant in the key/value axis j), so the
SPMD program always works on columns [0, 2048).

Numerics: matmul operands in bf16 (PE streams 1 column/cycle), fp32
PSUM accumulation everywhere, softmax row-sums in fp32. GroupNorm is
never materialized: its affine (xn = A*x + B, A/B fp32 from bf16-x
stats) is folded into the projection weights on device:
    W @ (A*x + B) + b  ==  (W . A_col) @ x + (W @ B + b)
Scores are bounded (|s|/16 <~ 1) so exp() skips max-subtraction.

Schedule: attention is software-pipelined with a 2-chunk lookahead
(chunk j's PV matmuls are emitted after chunk j+2's score matmuls) so
the in-order PE queue never waits on the ACT exp; each query-tile's
softmax/output tail is split in two and deferred into the next tile's
early chunks.
"""

import sys

sys.path.insert(0, "/opt/trn_rl_repo")

import numpy as np

B, C, N = 4, 256, 4096
HALF = N // 2
P = 128
NCORES = 8
GROUPS = 32
GSIZE = C // GROUPS  # 8
EPS = 1e-6
SCALE = C ** (-0.5)  # 1/16
ITILE = 512  # query-tile width
NIT = HALF // ITILE  # 4 query tiles per core
NJC = N // P  # 32 key chunks

_PROG = None
_LAST_RESULTS = None
_TRACE = False


def _build():
    import concourse.bass as bass
    import concourse.tile as tile
    from concourse import bacc, mybir

    F32 = mybir.dt.float32
    F32R = mybir.dt.float32r
    BF16 = mybir.dt.bfloat16
    FP8 = mybir.dt.float8e4
    DR = mybir.MatmulPerfMode.DoubleRow
    AF = mybir.ActivationFunctionType
    OP = mybir.AluOpType

    nc = bacc.Bacc("TRN2", target_bir_lowering=False, debug=False,
                   num_devices=NCORES)

    xbf_d = nc.declare_dram_parameter("xbf", [C, N], BF16, isOutput=False)
    x8_d = nc.declare_dram_parameter("x8", [C, N], FP8, isOutput=False)
    xres_d = nc.declare_dram_parameter("xres", [C, HALF], F32, isOutput=False)
    wq_d = nc.declare_dram_parameter("wqT", [C, C], BF16, isOutput=False)
    wk_d = nc.declare_dram_parameter("wkT", [C, C], BF16, isOutput=False)
    wv_d = nc.declare_dram_parameter("wvT", [C, C], BF16, isOutput=False)
    wo_d = nc.declare_dram_parameter("woT", [C, C], BF16, isOutput=False)
    bq_d = nc.declare_dram_parameter("bq", [C], F32, isOutput=False)
    bk_d = nc.declare_dram_parameter("bk", [C], F32, isOutput=False)
    bv_d = nc.declare_dram_parameter("bv", [C], F32, isOutput=False)
    bo_d = nc.declare_dram_parameter("bo", [C], F32, isOutput=False)
    gnw_d = nc.declare_dram_parameter("gnw", [C], F32, isOutput=False)
    gnb_d = nc.declare_dram_parameter("gnb", [C], F32, isOutput=False)
    a8_d = nc.declare_dram_parameter("a8", [C, GROUPS], F32, isOutput=False)
    e8_d = nc.declare_dram_parameter("e8", [P, C], F32, isOutput=False)
    out_d = nc.declare_dram_parameter("out", [C, HALF], F32, isOutput=True)

    with tile.TileContext(nc) as tc:
        with (
            tc.tile_pool(name="big", bufs=1) as big,
            tc.tile_pool(name="small", bufs=1) as small,
            tc.tile_pool(name="pp", bufs=5) as pp,
            tc.tile_pool(name="tip", bufs=3) as tip,
            tc.tile_pool(name="accp", bufs=2) as accp,
            tc.tile_pool(name="op", bufs=2) as op_pool,
            tc.tile_pool(name="resp", bufs=3) as resp,
            tc.tile_pool(name="rp", bufs=2) as rp,
            tc.tile_pool(name="psS", bufs=2, space="PSUM") as psS,
            tc.tile_pool(name="psO", bufs=1, space="PSUM") as psO,
            tc.tile_pool(name="psL", bufs=2, space="PSUM") as psL,
        ):
            # ---- load inputs ----
            # x slices go first on both DMA queues (their completion sems
            # must not be shared with later DMAs, which would fake-delay the
            # stats ops); constants, weights, then x8 follow.
            x_sb = big.tile([P, 2, N], BF16, tag="x")
            x_re = xbf_d[:].rearrange("(o p) j -> p o j", p=P)
            for o in range(2):
                for s in range(4):
                    eng = nc.sync if (s % 2 == 0) else nc.gpsimd
                    eng.dma_start(out=x_sb[:, o, s * 1024:(s + 1) * 1024],
                                  in_=x_re[:, o, s * 1024:(s + 1) * 1024])

            bq_sb = small.tile([P, 2], F32, tag="bq")
            bk_sb = small.tile([P, 2], F32, tag="bk")
            bo_sb = small.tile([P, 2], F32, tag="bo")
            gnw_sb = small.tile([P, 2], F32, tag="gnw")
            gnb_sb = small.tile([P, 2], F32, tag="gnb")
            for t, d in [(bq_sb, bq_d), (bk_sb, bk_d), (bo_sb, bo_d),
                         (gnw_sb, gnw_d), (gnb_sb, gnb_d)]:
                nc.sync.dma_start(out=t, in_=d[:].rearrange("(o p) -> p o", p=P))
            a8_sb = small.tile([P, 2, GROUPS], F32, tag="a8")
            nc.sync.dma_start(out=a8_sb, in_=a8_d[:].rearrange("(o p) g -> p o g", p=P))
            e8_sb = small.tile([P, 2, P], F32, tag="e8")
            nc.sync.dma_start(out=e8_sb, in_=e8_d[:].rearrange("g (o m) -> g o m", m=P))

            wq_sb = small.tile([P, 2, C], BF16, tag="wq")
            wk_sb = small.tile([P, 2, C], BF16, tag="wk")
            wv_sb = small.tile([P, 2, C], BF16, tag="wv")
            wo_sb = small.tile([P, 2, C], BF16, tag="wo")
            for t, d in [(wq_sb, wq_d), (wk_sb, wk_d), (wv_sb, wv_d), (wo_sb, wo_d)]:
                nc.gpsimd.dma_start(out=t, in_=d[:].rearrange("(o p) c -> p o c", p=P))

            x8_sb = big.tile([P, 2, N], FP8, tag="x8")
            x8_re = x8_d[:].rearrange("(o p) j -> p o j", p=P)
            for o in range(2):
                for h in range(2):
                    eng = nc.sync if h == 0 else nc.gpsimd
                    eng.dma_start(out=x8_sb[:, o, h * 2048:(h + 1) * 2048],
                                  in_=x8_re[:, o, h * 2048:(h + 1) * 2048])

            ones_f32 = small.tile([P, 2, P], F32, tag="onesf")
            nc.vector.memset(ones_f32, 1.0)
            ones8 = small.tile([P, 2, P], FP8, tag="ones8")
            nc.vector.tensor_copy(out=ones8, in_=ones_f32)
            eps_t = small.tile([P, 1], F32, tag="eps")
            nc.vector.memset(eps_t, EPS)

            def ps1():
                t = psS.tile([P, 2, ITILE], F32, tag="ps", name="ps1b")
                return t[:, 0, :]

            # ---- GroupNorm stats (fp32 accumulation over bf16 x) ----
            # per-channel sum on DVE (tensor_reduce) and sum-of-squares on
            # ACT (Square + accum_out), one 1024-column slice each, so the
            # two engines chew through the stats in parallel with the DMA
            rhs_stats = small.tile([P, 2, 2], F32, tag="rhs_stats")
            sq_junk = small.tile([P, 1024], BF16, tag="sqjunk")
            for o in range(2):
                sums = small.tile([P, 4], F32, tag=f"sums{o}")
                sqs = small.tile([P, 4], F32, tag=f"sqs{o}")
                for s in range(4):
                    sl = x_sb[:, o, s * 1024:(s + 1) * 1024]
                    nc.vector.tensor_reduce(out=sums[:, s:s + 1], in_=sl,
                                            axis=mybir.AxisListType.X,
                                            op=OP.add)
                    nc.scalar.activation(out=sq_junk, in_=sl, func=AF.Square,
                                         accum_out=sqs[:, s:s + 1])
                pair = small.tile([P, 2, 2], F32, tag=f"pair{o}")
                nc.vector.tensor_tensor(out=pair[:, 0], in0=sums[:, 0:2],
                                        in1=sums[:, 2:4], op=OP.add)
                nc.vector.tensor_tensor(out=pair[:, 1], in0=sqs[:, 0:2],
                                        in1=sqs[:, 2:4], op=OP.add)
                tot = small.tile([P, 2], F32, tag=f"tot{o}")
                nc.vector.tensor_tensor(out=tot, in0=pair[:, :, 0],
                                        in1=pair[:, :, 1], op=OP.add)
                # rhs_stats[:, o, 0] = mean_c ; rhs_stats[:, o, 1] = E_c[x^2]
                nc.vector.tensor_scalar_mul(out=rhs_stats[:, o], in0=tot,
                                            scalar1=1.0 / N)

            # pool 8 channels -> 32 groups:  [32, 2] = a8^T @ rhs_stats
            g_ps = ps1()
            nc.tensor.matmul(g_ps[0:GROUPS, 0:2], lhsT=a8_sb[:, 0], rhs=rhs_stats[:, 0],
                             start=True, stop=False)
            nc.tensor.matmul(g_ps[0:GROUPS, 0:2], lhsT=a8_sb[:, 1], rhs=rhs_stats[:, 1],
                             start=False, stop=True)
            # stats32[:, 0] = group mean, stats32[:, 1] = group rstd
            gsb = small.tile([P, 2], F32, tag="gsb")
            nc.vector.tensor_copy(out=gsb[0:GROUPS], in_=g_ps[0:GROUPS, 0:2])
            stats32 = small.tile([P, 2], F32, tag="stats32")
            nc.vector.memset(stats32, 0.0)
            nc.vector.tensor_copy(out=stats32[0:GROUPS, 0:1], in_=gsb[0:GROUPS, 0:1])
            gm2 = small.tile([P, 1], F32, tag="gm2")
            nc.vector.tensor_mul(out=gm2[0:GROUPS], in0=gsb[0:GROUPS, 0:1],
                                 in1=gsb[0:GROUPS, 0:1])
            gvar = small.tile([P, 1], F32, tag="gvar")
            nc.vector.tensor_sub(out=gvar[0:GROUPS], in0=gsb[0:GROUPS, 1:2],
                                 in1=gm2[0:GROUPS])
            gsd = small.tile([P, 1], F32, tag="gsd")
            nc.scalar.activation(out=gsd[0:GROUPS], in_=gvar[0:GROUPS], func=AF.Sqrt,
                                 bias=eps_t[0:GROUPS], scale=1.0)
            nc.vector.reciprocal(out=stats32[0:GROUPS, 1:2], in_=gsd[0:GROUPS])

            # expand 32 groups -> 256 channels, fold in gn affine:
            #   A_c = rstd_g(c) * gn_w_c ;  B_c = gn_b_c - mean_g(c) * A_c
            A_t = small.tile([P, 2], F32, tag="A")
            B_t = small.tile([P, 2], F32, tag="Bt")
            for o in range(2):
                e_ps = ps1()
                nc.tensor.matmul(e_ps[:, 0:2], lhsT=e8_sb[:, o], rhs=stats32,
                                 start=True, stop=True)
                nc.vector.tensor_mul(out=A_t[:, o:o + 1], in0=e_ps[:, 1:2],
                                     in1=gnw_sb[:, o:o + 1])
                mA = small.tile([P, 1], F32, tag=f"mA{o}")
                nc.vector.tensor_mul(out=mA, in0=e_ps[:, 0:1], in1=A_t[:, o:o + 1])
                nc.vector.tensor_sub(out=B_t[:, o:o + 1], in0=gnb_sb[:, o:o + 1], in1=mA)
            B_bf = small.tile([P, 2], BF16, tag="Bbf")
            nc.vector.tensor_copy(out=B_bf, in_=B_t)

            # ---- fold GN affine into projection weights + biases ----
            # folded bias:  b' = W @ B + b   (uses the unscaled weights).
            # The V bias is not applied to V^T at all: since
            # out = Wo @ (V P^T / l) + ..., a constant column bias bv' on V
            # contributes exactly Wo @ bv' to every output column, so it is
            # folded into the output-projection bias bo2 instead.
            bq2 = small.tile([P, 2], F32, tag="bq2")
            bk2 = small.tile([P, 2], F32, tag="bk2")
            bv2 = small.tile([P, 2], BF16, tag="bv2")
            for (w_sb, b_sb, b2) in [(wq_sb, bq_sb, bq2), (wk_sb, bk_sb, bk2)]:
                for oo in range(2):
                    bps = ps1()
                    for ci in range(2):
                        nc.tensor.matmul(bps[:, 0:1],
                                         lhsT=w_sb[:, ci, oo * P:(oo + 1) * P],
                                         rhs=B_bf[:, ci:ci + 1],
                                         start=(ci == 0), stop=(ci == 1))
                    nc.vector.tensor_add(out=b2[:, oo:oo + 1], in0=bps[:, 0:1],
                                         in1=b_sb[:, oo:oo + 1])
            bv_sb = small.tile([P, 2], F32, tag="bvp")
            nc.sync.dma_start(out=bv_sb, in_=bv_d[:].rearrange("(o p) -> p o", p=P))
            for oo in range(2):
                bps = ps1()
                for ci in range(2):
                    nc.tensor.matmul(bps[:, 0:1],
                                     lhsT=wv_sb[:, ci, oo * P:(oo + 1) * P],
                                     rhs=B_bf[:, ci:ci + 1],
                                     start=(ci == 0), stop=(ci == 1))
                nc.vector.tensor_add(out=bv2[:, oo:oo + 1], in0=bps[:, 0:1],
                                     in1=bv_sb[:, oo:oo + 1])
            # bo2 = bo + Wo @ bv'
            bo2 = small.tile([P, 2], F32, tag="bo2")
            for oo in range(2):
                bps = ps1()
                for ci in range(2):
                    nc.tensor.matmul(bps[:, 0:1],
                                     lhsT=wo_sb[:, ci, oo * P:(oo + 1) * P],
                                     rhs=bv2[:, ci:ci + 1],
                                     start=(ci == 0), stop=(ci == 1))
                nc.vector.tensor_add(out=bo2[:, oo:oo + 1], in0=bps[:, 0:1],
                                     in1=bo_sb[:, oo:oo + 1])

            # fp8 copies of the projection weights with the GN scale folded:
            #   W8T[c', o] = WT[c', o] * A[c']
            wq8 = small.tile([P, 2, C], FP8, tag="wq8")
            wk8 = small.tile([P, 2, C], FP8, tag="wk8")
            wv8 = small.tile([P, 2, C], FP8, tag="wv8")
            for w_sb, w8 in [(wq_sb, wq8), (wk_sb, wk8), (wv_sb, wv8)]:
                for ci in range(2):
                    nc.vector.tensor_scalar_mul(out=w8[:, ci], in0=w_sb[:, ci],
                                                scalar1=A_t[:, ci:ci + 1])

            # ---- projections (fp8 DoubleRow: full C=256 contraction/MM) ----
            # PSUMs round-robin over four pools and evacuations alternate
            # ACT / DVE, so the PSUM slot round-trips overlap ~4 deep
            k_sb = big.tile([P, 2, N], FP8, tag="k")
            q_sb = big.tile([P, 2, HALF], FP8, tag="q")
            v_sb = big.tile([P, NJC, C], FP8, tag="v")
            for jt in range(N // ITILE):
                jts = slice(jt * ITILE, (jt + 1) * ITILE)
                for oo in range(2):
                    k_ps = psO.tile([P, ITILE], F32, tag=("o0" if oo == 0 else "o1"),
                                    name=f"kps{jt}_{oo}")
                    nc.tensor.matmul(k_ps, lhsT=wk8[:, :, oo * P:(oo + 1) * P],
                                     rhs=x8_sb[:, :, jts],
                                     perf_mode=DR, start=True, stop=True)
                    if oo == 0:
                        nc.scalar.activation(out=k_sb[:, oo, jts],
                                             in_=k_ps, func=AF.Identity,
                                             bias=bk2[:, oo:oo + 1], scale=1.0)
                    else:
                        nc.vector.tensor_scalar_add(out=k_sb[:, oo, jts],
                                                    in0=k_ps,
                                                    scalar1=bk2[:, oo:oo + 1])
                if jt < NIT:
                    # Q: only my 2048 query columns (always columns [0, 2048))
                    for oo in range(2):
                        q_ps = ps1()
                        nc.tensor.matmul(q_ps, lhsT=wq8[:, :, oo * P:(oo + 1) * P],
                                         rhs=x8_sb[:, :, jts],
                                         perf_mode=DR, start=True, stop=True)
                        if oo == 0:
                            nc.scalar.activation(out=q_sb[:, oo, jts],
                                                 in_=q_ps, func=AF.Identity,
                                                 bias=bq2[:, oo:oo + 1], scale=1.0)
                        else:
                            nc.vector.tensor_scalar_add(out=q_sb[:, oo, jts],
                                                        in0=q_ps,
                                                        scalar1=bq2[:, oo:oo + 1])
                # V^T for this tile's four j-chunks: [j on partitions, c free];
                # no bias (folded into bo2), evacuation is a pure copy
                for jc in range(4 * jt, 4 * jt + 4):
                    v_ps = psL.tile([P, ITILE], F32, tag="lps", name=f"vps{jc}")
                    nc.tensor.matmul(v_ps[:, 0:C],
                                     lhsT=x8_sb[:, :, jc * P:(jc + 1) * P],
                                     rhs=wv8[:, :, :],
                                     perf_mode=DR, start=True, stop=True)
                    if jc % 2 == 0:
                        nc.scalar.activation(out=v_sb[:, jc], in_=v_ps[:, 0:C],
                                             func=AF.Copy)
                    else:
                        nc.vector.tensor_copy(out=v_sb[:, jc], in_=v_ps[:, 0:C])

            # residual input, only needed from the first attention tail on
            xres_sb = big.tile([P, 2, HALF], F32, tag="xres")
            nc.sync.dma_start(out=xres_sb, in_=xres_d[:].rearrange("(o p) i -> p o i", p=P))

            # ---- attention (fp8 DoubleRow, software-pipelined in chunk pairs) ----
            NPR = NJC // 2  # 16 key-chunk pairs

            # Schraudolph fast-exp constants (DVE path): for scores s,
            # exp(s*SCALE) ~= bitcast_f32(int32(s*SA + SB)); its ~3% sawtooth
            # error is below the fp8 quantization noise already on P.
            SA = SCALE * 1.4426950408889634 * (1 << 23)
            SB = 127.0 * (1 << 23) - 366000.0 + 0.5
            I32 = mybir.dt.int32

            def emit_s_exp(it, pr, use_dve):
                """Scores for chunk pair (2pr, 2pr+1) and P = exp(S*scale).

                Each DoubleRow matmul contracts the full C=256 via the fp8
                k-interleave; one exp op covers both chunks (2 PSUM banks).
                exp runs on ACT (exact) or DVE (Schraudolph) to split the
                transcendental load across both engines.
                """
                isl = slice(it * ITILE, (it + 1) * ITILE)
                s_ps = psS.tile([P, 2, ITILE], F32, tag="ps")
                for par in range(2):
                    jc = 2 * pr + par
                    jsl = slice(jc * P, (jc + 1) * P)
                    nc.tensor.matmul(s_ps[:, par, :], lhsT=k_sb[:, :, jsl],
                                     rhs=q_sb[:, :, isl],
                                     perf_mode=DR, start=True, stop=True)
                p2 = pp.tile([P, 2, ITILE], FP8, tag="p")
                if use_dve:
                    ti = tip.tile([P, 2, ITILE], I32, tag="ti")
                    nc.vector.tensor_scalar(out=ti, in0=s_ps, scalar1=SA,
                                            scalar2=SB, op0=OP.mult, op1=OP.add)
                    nc.vector.tensor_copy(out=p2, in_=ti.bitcast(F32))
                else:
                    nc.scalar.activation(out=p2, in_=s_ps, func=AF.Exp, scale=SCALE)
                return p2

            def emit_pv(pr, p2, o_ps0, o_ps1, l_ps):
                st, sp = (pr == 0), (pr == NPR - 1)
                nc.tensor.matmul(o_ps0, lhsT=v_sb[:, 2 * pr:2 * pr + 2, 0:P],
                                 rhs=p2, perf_mode=DR, start=st, stop=sp)
                nc.tensor.matmul(o_ps1, lhsT=v_sb[:, 2 * pr:2 * pr + 2, P:C],
                                 rhs=p2, perf_mode=DR, start=st, stop=sp)
                nc.tensor.matmul(l_ps, lhsT=ones8, rhs=p2,
                                 perf_mode=DR, start=st, stop=sp)

            def make_tail_a(it, o_ps0, o_ps1, l_ps):
                def tail_a():
                    recip = rp.tile([P, ITILE], F32, tag="recip")
                    nc.vector.reciprocal_approx_fast(out=recip, in_=l_ps)
                    o_sb = op_pool.tile([P, 2, ITILE], BF16, tag="osb")
                    nc.vector.tensor_tensor(out=o_sb[:, 0], in0=o_ps0,
                                            in1=recip, op=OP.mult)
                    nc.vector.tensor_tensor(out=o_sb[:, 1], in0=o_ps1,
                                            in1=recip, op=OP.mult)
                    return o_sb
                return tail_a

            def make_tail_b(it, o_sb):
                def tail_b():
                    isl = slice(it * ITILE, (it + 1) * ITILE)
                    # output projection + bias + residual
                    for oo in range(2):
                        u_ps = ps1()
                        for ci in range(2):
                            nc.tensor.matmul(
                                u_ps, lhsT=wo_sb[:, ci, oo * P:(oo + 1) * P],
                                rhs=o_sb[:, ci],
                                start=(ci == 0), stop=(ci == 1))
                        res = resp.tile([P, ITILE], F32, tag="res")
                        nc.vector.scalar_tensor_tensor(
                            out=res, in0=u_ps, scalar=bo2[:, oo:oo + 1],
                            in1=xres_sb[:, oo, isl], op0=OP.add, op1=OP.add)
                        nc.sync.dma_start(
                            out=out_d[:].rearrange("(o p) i -> p o i", p=P)[:, oo, isl],
                            in_=res)
                return tail_b

            pending_a = pending_b = None
            for it in range(NIT):
                o_ps0 = psO.tile([P, ITILE], F32, tag="o0")
                o_ps1 = psO.tile([P, ITILE], F32, tag="o1")
                l_ps = psL.tile([P, ITILE], F32, tag="lps")
                pq = []
                for pr in range(NPR):
                    pq.append(emit_s_exp(it, pr, use_dve=False))
                    if pr >= 2:
                        emit_pv(pr - 2, pq[pr - 2], o_ps0, o_ps1, l_ps)
                    if pending_a is not None and pr == 2:
                        o_sb_prev = pending_a()
                        pending_b = make_tail_b(it - 1, o_sb_prev)
                        pending_a = None
                    if pending_b is not None and pr == 5:
                        pending_b()
                        pending_b = None
                for pr in (NPR - 2, NPR - 1):
                    emit_pv(pr, pq[pr], o_ps0, o_ps1, l_ps)
                pending_a = make_tail_a(it, o_ps0, o_ps1, l_ps)
            o_sb_last = pending_a()
            make_tail_b(NIT - 1, o_sb_last)()

    nc.compile()
    return nc


def _get_prog():
    global _PROG
    if _PROG is None:
        _PROG = _build()
    return _PROG


def kernel(x, gn_w, gn_b, wq, bq, wk, bk, wv, bv, wo, bo):
    global _LAST_RESULTS
    import ml_dtypes
    from concourse.bass_utils import run_bass_kernel_spmd

    nc = _get_prog()

    f32 = lambda a: np.ascontiguousarray(np.asarray(a), dtype=np.float32)
    bf16 = lambda a: np.ascontiguousarray(np.asarray(a, dtype=np.float32).astype(ml_dtypes.bfloat16))
    fp8 = lambda a: np.ascontiguousarray(np.asarray(a, dtype=np.float32).astype(ml_dtypes.float8_e4m3fn))
    x = f32(x).reshape(B, C, N)
    shared = {
        "wqT": bf16(np.asarray(wq).T), "wkT": bf16(np.asarray(wk).T),
        "wvT": bf16(np.asarray(wv).T), "woT": bf16(np.asarray(wo).T),
        "bq": f32(bq), "bk": f32(bk), "bv": f32(bv), "bo": f32(bo),
        "gnw": f32(gn_w), "gnb": f32(gn_b),
    }
    a8 = np.zeros((C, GROUPS), np.float32)
    a8[np.arange(C), np.arange(C) // GSIZE] = 1.0 / GSIZE
    shared["a8"] = a8
    e8 = np.zeros((P, C), np.float32)
    e8[np.arange(C) // GSIZE, np.arange(C)] = 1.0
    shared["e8"] = e8

    in_maps = []
    for core in range(NCORES):
        b, h = core // 2, core % 2
        xb = x[b]
        if h == 0:
            xc = xb
        else:
            xc = np.ascontiguousarray(np.concatenate([xb[:, HALF:], xb[:, :HALF]], axis=1))
        m = dict(shared)
        m["xbf"] = bf16(xc)
        m["x8"] = fp8(xc)
        m["xres"] = np.ascontiguousarray(xb[:, h * HALF:(h + 1) * HALF])
        in_maps.append(m)

    _LAST_RESULTS = run_bass_kernel_spmd(nc, in_maps, list(range(NCORES)),
                                         trace=_TRACE)
    out = np.empty((B, C, N), np.float32)
    for core in range(NCORES):
        b, h = core // 2, core % 2
        out[b, :, h * HALF:(h + 1) * HALF] = _LAST_RESULTS.results[core]["out"]
    return out.reshape(B, C, 16, 16, 16)


# revision 28
# speedup vs baseline: 1.0319x; 1.0319x over previous
"""Trainium2 Bass kernel for nn_AttentionBlock (GroupNorm + 1-head self-attention).

Reference computation (per batch b, C=256 channels, N=4096 spatial):
    xn = GroupNorm(x; 32 groups, eps=1e-6) * gn_w + gn_b
    q/k/v = W @ xn + b          (1x1 conv == channel matmul)
    attn  = softmax(q^T k / 16, axis=j)
    out   = x + Wo @ (v @ attn^T) + bo

Sharding: 8 cores = 4 batches x 2 query-halves. Each core computes
GroupNorm + K/V for its whole batch (duplicated across the pair) and
attention + output projection for its 2048 query rows.

Per-core x is sent with its own query columns rotated to the front
(attention is permutation-equivariant in the key/value axis j), so the
SPMD program always works on columns [0, 2048).

Numerics: matmul operands in bf16 (PE streams 1 column/cycle), fp32
PSUM accumulation everywhere, softmax row-sums in fp32. GroupNorm is
never materialized: its affine (xn = A*x + B, A/B fp32 from bf16-x
stats) is folded into the projection weights on device:
    W @ (A*x + B) + b  ==  (W . A_col) @ x + (W @ B + b)
Scores are bounded (|s|/16 <~ 1) so exp() skips max-subtraction.

Schedule: attention is software-pipelined with a 2-chunk lookahead
(chunk j's PV matmuls are emitted after chunk j+2's score matmuls) so
the in-order PE queue never waits on the ACT exp; each query-tile's
softmax/output tail is split in two and deferred into the next tile's
early chunks.
"""

import sys

sys.path.insert(0, "/opt/trn_rl_repo")

import numpy as np

B, C, N = 4, 256, 4096
HALF = N // 2
P = 128
NCORES = 8
GROUPS = 32
GSIZE = C // GROUPS  # 8
EPS = 1e-6
SCALE = C ** (-0.5)  # 1/16
ITILE = 512  # query-tile width
NIT = HALF // ITILE  # 4 query tiles per core
NJC = N // P  # 32 key chunks

_PROG = None
_LAST_RESULTS = None
_TRACE = False


def _build():
    import concourse.bass as bass
    import concourse.tile as tile
    from concourse import bacc, mybir

    F32 = mybir.dt.float32
    F32R = mybir.dt.float32r
    BF16 = mybir.dt.bfloat16
    FP8 = mybir.dt.float8e4
    DR = mybir.MatmulPerfMode.DoubleRow
    AF = mybir.ActivationFunctionType
    OP = mybir.AluOpType

    nc = bacc.Bacc("TRN2", target_bir_lowering=False, debug=False,
                   num_devices=NCORES)

    xbf_d = nc.declare_dram_parameter("xbf", [C, N], BF16, isOutput=False)
    x8_d = nc.declare_dram_parameter("x8", [C, N], FP8, isOutput=False)
    xres_d = nc.declare_dram_parameter("xres", [C, HALF], F32, isOutput=False)
    wq_d = nc.declare_dram_parameter("wqT", [C, C], BF16, isOutput=False)
    wk_d = nc.declare_dram_parameter("wkT", [C, C], BF16, isOutput=False)
    wv_d = nc.declare_dram_parameter("wvT", [C, C], BF16, isOutput=False)
    wo_d = nc.declare_dram_parameter("woT", [C, C], BF16, isOutput=False)
    bq_d = nc.declare_dram_parameter("bq", [C], F32, isOutput=False)
    bk_d = nc.declare_dram_parameter("bk", [C], F32, isOutput=False)
    bv_d = nc.declare_dram_parameter("bv", [C], F32, isOutput=False)
    bo_d = nc.declare_dram_parameter("bo", [C], F32, isOutput=False)
    gnw_d = nc.declare_dram_parameter("gnw", [C], F32, isOutput=False)
    gnb_d = nc.declare_dram_parameter("gnb", [C], F32, isOutput=False)
    a8_d = nc.declare_dram_parameter("a8", [C, GROUPS], F32, isOutput=False)
    e8_d = nc.declare_dram_parameter("e8", [P, C], F32, isOutput=False)
    out_d = nc.declare_dram_parameter("out", [C, HALF], F32, isOutput=True)

    with tile.TileContext(nc) as tc:
        with (
            tc.tile_pool(name="big", bufs=1) as big,
            tc.tile_pool(name="small", bufs=1) as small,
            tc.tile_pool(name="pp", bufs=5) as pp,
            tc.tile_pool(name="tip", bufs=3) as tip,
            tc.tile_pool(name="accp", bufs=2) as accp,
            tc.tile_pool(name="op", bufs=2) as op_pool,
            tc.tile_pool(name="resp", bufs=3) as resp,
            tc.tile_pool(name="rp", bufs=2) as rp,
            tc.tile_pool(name="psS", bufs=2, space="PSUM") as psS,
            tc.tile_pool(name="psO", bufs=1, space="PSUM") as psO,
            tc.tile_pool(name="psL", bufs=2, space="PSUM") as psL,
        ):
            # ---- load inputs ----
            # x slices go first on both DMA queues (their completion sems
            # must not be shared with later DMAs, which would fake-delay the
            # stats ops); constants, weights, then x8 follow.
            x_sb = big.tile([P, 2, N], BF16, tag="x")
            x_re = xbf_d[:].rearrange("(o p) j -> p o j", p=P)
            for o in range(2):
                for s in range(4):
                    eng = nc.sync if (s % 2 == 0) else nc.gpsimd
                    eng.dma_start(out=x_sb[:, o, s * 1024:(s + 1) * 1024],
                                  in_=x_re[:, o, s * 1024:(s + 1) * 1024])

            bq_sb = small.tile([P, 2], F32, tag="bq")
            bk_sb = small.tile([P, 2], F32, tag="bk")
            bo_sb = small.tile([P, 2], F32, tag="bo")
            gnw_sb = small.tile([P, 2], F32, tag="gnw")
            gnb_sb = small.tile([P, 2], F32, tag="gnb")
            for t, d in [(bq_sb, bq_d), (bk_sb, bk_d), (bo_sb, bo_d),
                         (gnw_sb, gnw_d), (gnb_sb, gnb_d)]:
                nc.sync.dma_start(out=t, in_=d[:].rearrange("(o p) -> p o", p=P))
            a8_sb = small.tile([P, 2, GROUPS], F32, tag="a8")
            nc.sync.dma_start(out=a8_sb, in_=a8_d[:].rearrange("(o p) g -> p o g", p=P))
            e8_sb = small.tile([P, 2, P], F32, tag="e8")
            nc.sync.dma_start(out=e8_sb, in_=e8_d[:].rearrange("g (o m) -> g o m", m=P))

            wq_sb = small.tile([P, 2, C], BF16, tag="wq")
            wk_sb = small.tile([P, 2, C], BF16, tag="wk")
            wv_sb = small.tile([P, 2, C], BF16, tag="wv")
            wo_sb = small.tile([P, 2, C], BF16, tag="wo")
            for t, d in [(wq_sb, wq_d), (wk_sb, wk_d), (wv_sb, wv_d), (wo_sb, wo_d)]:
                nc.gpsimd.dma_start(out=t, in_=d[:].rearrange("(o p) c -> p o c", p=P))

            x8_sb = big.tile([P, 2, N], FP8, tag="x8")
            x8_re = x8_d[:].rearrange("(o p) j -> p o j", p=P)
            for o in range(2):
                for h in range(2):
                    eng = nc.sync if h == 0 else nc.gpsimd
                    eng.dma_start(out=x8_sb[:, o, h * 2048:(h + 1) * 2048],
                                  in_=x8_re[:, o, h * 2048:(h + 1) * 2048])

            ones_f32 = small.tile([P, 2, P], F32, tag="onesf")
            nc.vector.memset(ones_f32, 1.0)
            ones8 = small.tile([P, 2, P], FP8, tag="ones8")
            nc.vector.tensor_copy(out=ones8, in_=ones_f32)
            eps_t = small.tile([P, 1], F32, tag="eps")
            nc.vector.memset(eps_t, EPS)

            def ps1():
                t = psS.tile([P, 2, ITILE], F32, tag="ps", name="ps1b")
                return t[:, 0, :]

            # ---- GroupNorm stats (fp32 accumulation over bf16 x) ----
            # per-channel sum on DVE (tensor_reduce) and sum-of-squares on
            # ACT (Square + accum_out), one 1024-column slice each, so the
            # two engines chew through the stats in parallel with the DMA
            rhs_stats = small.tile([P, 2, 2], F32, tag="rhs_stats")
            sq_junk = small.tile([P, 1024], BF16, tag="sqjunk")
            for o in range(2):
                sums = small.tile([P, 4], F32, tag=f"sums{o}")
                sqs = small.tile([P, 4], F32, tag=f"sqs{o}")
                for s in range(4):
                    sl = x_sb[:, o, s * 1024:(s + 1) * 1024]
                    nc.vector.tensor_reduce(out=sums[:, s:s + 1], in_=sl,
                                            axis=mybir.AxisListType.X,
                                            op=OP.add)
                    nc.scalar.activation(out=sq_junk, in_=sl, func=AF.Square,
                                         accum_out=sqs[:, s:s + 1])
                pair = small.tile([P, 2, 2], F32, tag=f"pair{o}")
                nc.vector.tensor_tensor(out=pair[:, 0], in0=sums[:, 0:2],
                                        in1=sums[:, 2:4], op=OP.add)
                nc.vector.tensor_tensor(out=pair[:, 1], in0=sqs[:, 0:2],
                                        in1=sqs[:, 2:4], op=OP.add)
                tot = small.tile([P, 2], F32, tag=f"tot{o}")
                nc.vector.tensor_tensor(out=tot, in0=pair[:, :, 0],
                                        in1=pair[:, :, 1], op=OP.add)
                # rhs_stats[:, o, 0] = mean_c ; rhs_stats[:, o, 1] = E_c[x^2]
                nc.vector.tensor_scalar_mul(out=rhs_stats[:, o], in0=tot,
                                            scalar1=1.0 / N)

            # pool 8 channels -> 32 groups:  [32, 2] = a8^T @ rhs_stats
            g_ps = ps1()
            nc.tensor.matmul(g_ps[0:GROUPS, 0:2], lhsT=a8_sb[:, 0], rhs=rhs_stats[:, 0],
                             start=True, stop=False)
            nc.tensor.matmul(g_ps[0:GROUPS, 0:2], lhsT=a8_sb[:, 1], rhs=rhs_stats[:, 1],
                             start=False, stop=True)
            # stats32[:, 0] = group mean, stats32[:, 1] = group rstd
            gsb = small.tile([P, 2], F32, tag="gsb")
            nc.vector.tensor_copy(out=gsb[0:GROUPS], in_=g_ps[0:GROUPS, 0:2])
            stats32 = small.tile([P, 2], F32, tag="stats32")
            nc.vector.memset(stats32, 0.0)
            nc.vector.tensor_copy(out=stats32[0:GROUPS, 0:1], in_=gsb[0:GROUPS, 0:1])
            gm2 = small.tile([P, 1], F32, tag="gm2")
            nc.vector.tensor_mul(out=gm2[0:GROUPS], in0=gsb[0:GROUPS, 0:1],
                                 in1=gsb[0:GROUPS, 0:1])
            gvar = small.tile([P, 1], F32, tag="gvar")
            nc.vector.tensor_sub(out=gvar[0:GROUPS], in0=gsb[0:GROUPS, 1:2],
                                 in1=gm2[0:GROUPS])
            gsd = small.tile([P, 1], F32, tag="gsd")
            nc.scalar.activation(out=gsd[0:GROUPS], in_=gvar[0:GROUPS], func=AF.Sqrt,
                                 bias=eps_t[0:GROUPS], scale=1.0)
            nc.vector.reciprocal(out=stats32[0:GROUPS, 1:2], in_=gsd[0:GROUPS])

            # expand 32 groups -> 256 channels, fold in gn affine:
            #   A_c = rstd_g(c) * gn_w_c ;  B_c = gn_b_c - mean_g(c) * A_c
            A_t = small.tile([P, 2], F32, tag="A")
            B_t = small.tile([P, 2], F32, tag="Bt")
            for o in range(2):
                e_ps = ps1()
                nc.tensor.matmul(e_ps[:, 0:2], lhsT=e8_sb[:, o], rhs=stats32,
                                 start=True, stop=True)
                nc.vector.tensor_mul(out=A_t[:, o:o + 1], in0=e_ps[:, 1:2],
                                     in1=gnw_sb[:, o:o + 1])
                mA = small.tile([P, 1], F32, tag=f"mA{o}")
                nc.vector.tensor_mul(out=mA, in0=e_ps[:, 0:1], in1=A_t[:, o:o + 1])
                nc.vector.tensor_sub(out=B_t[:, o:o + 1], in0=gnb_sb[:, o:o + 1], in1=mA)
            B_bf = small.tile([P, 2], BF16, tag="Bbf")
            nc.vector.tensor_copy(out=B_bf, in_=B_t)

            # ---- fold GN affine into projection weights + biases ----
            # folded bias:  b' = W @ B + b   (uses the unscaled weights).
            # The V bias is not applied to V^T at all: since
            # out = Wo @ (V P^T / l) + ..., a constant column bias bv' on V
            # contributes exactly Wo @ bv' to every output column, so it is
            # folded into the output-projection bias bo2 instead.
            bq2 = small.tile([P, 2], F32, tag="bq2")
            bk2 = small.tile([P, 2], F32, tag="bk2")
            bv2 = small.tile([P, 2], BF16, tag="bv2")
            for (w_sb, b_sb, b2) in [(wq_sb, bq_sb, bq2), (wk_sb, bk_sb, bk2)]:
                for oo in range(2):
                    bps = ps1()
                    for ci in range(2):
                        nc.tensor.matmul(bps[:, 0:1],
                                         lhsT=w_sb[:, ci, oo * P:(oo + 1) * P],
                                         rhs=B_bf[:, ci:ci + 1],
                                         start=(ci == 0), stop=(ci == 1))
                    nc.vector.tensor_add(out=b2[:, oo:oo + 1], in0=bps[:, 0:1],
                                         in1=b_sb[:, oo:oo + 1])
            bv_sb = small.tile([P, 2], F32, tag="bvp")
            nc.sync.dma_start(out=bv_sb, in_=bv_d[:].rearrange("(o p) -> p o", p=P))
            for oo in range(2):
                bps = ps1()
                for ci in range(2):
                    nc.tensor.matmul(bps[:, 0:1],
                                     lhsT=wv_sb[:, ci, oo * P:(oo + 1) * P],
                                     rhs=B_bf[:, ci:ci + 1],
                                     start=(ci == 0), stop=(ci == 1))
                nc.vector.tensor_add(out=bv2[:, oo:oo + 1], in0=bps[:, 0:1],
                                     in1=bv_sb[:, oo:oo + 1])
            # bo2 = bo + Wo @ bv'
            bo2 = small.tile([P, 2], F32, tag="bo2")
            for oo in range(2):
                bps = ps1()
                for ci in range(2):
                    nc.tensor.matmul(bps[:, 0:1],
                                     lhsT=wo_sb[:, ci, oo * P:(oo + 1) * P],
                                     rhs=bv2[:, ci:ci + 1],
                                     start=(ci == 0), stop=(ci == 1))
                nc.vector.tensor_add(out=bo2[:, oo:oo + 1], in0=bps[:, 0:1],
                                     in1=bo_sb[:, oo:oo + 1])

            # fp8 copies of the projection weights with the GN scale folded:
            #   W8T[c', o] = WT[c', o] * A[c']
            wq8 = small.tile([P, 2, C], FP8, tag="wq8")
            wk8 = small.tile([P, 2, C], FP8, tag="wk8")
            wv8 = small.tile([P, 2, C], FP8, tag="wv8")
            for w_sb, w8 in [(wq_sb, wq8), (wk_sb, wk8), (wv_sb, wv8)]:
                for ci in range(2):
                    nc.vector.tensor_scalar_mul(out=w8[:, ci], in0=w_sb[:, ci],
                                                scalar1=A_t[:, ci:ci + 1])

            # ---- projections (fp8 DoubleRow: full C=256 contraction/MM) ----
            # PSUMs round-robin over four pools and evacuations alternate
            # ACT / DVE, so the PSUM slot round-trips overlap ~4 deep
            k_sb = big.tile([P, 2, N], FP8, tag="k")
            q_sb = big.tile([P, 2, HALF], FP8, tag="q")
            v_sb = big.tile([P, NJC, C], FP8, tag="v")
            for jt in range(N // ITILE):
                jts = slice(jt * ITILE, (jt + 1) * ITILE)
                for oo in range(2):
                    k_ps = psO.tile([P, ITILE], F32, tag=("o0" if oo == 0 else "o1"),
                                    name=f"kps{jt}_{oo}")
                    nc.tensor.matmul(k_ps, lhsT=wk8[:, :, oo * P:(oo + 1) * P],
                                     rhs=x8_sb[:, :, jts],
                                     perf_mode=DR, start=True, stop=True)
                    if oo == 0:
                        nc.scalar.activation(out=k_sb[:, oo, jts],
                                             in_=k_ps, func=AF.Identity,
                                             bias=bk2[:, oo:oo + 1], scale=1.0)
                    else:
                        nc.vector.tensor_scalar_add(out=k_sb[:, oo, jts],
                                                    in0=k_ps,
                                                    scalar1=bk2[:, oo:oo + 1])
                if jt < NIT:
                    # Q: only my 2048 query columns (always columns [0, 2048))
                    for oo in range(2):
                        q_ps = ps1()
                        nc.tensor.matmul(q_ps, lhsT=wq8[:, :, oo * P:(oo + 1) * P],
                                         rhs=x8_sb[:, :, jts],
                                         perf_mode=DR, start=True, stop=True)
                        if oo == 0:
                            nc.scalar.activation(out=q_sb[:, oo, jts],
                                                 in_=q_ps, func=AF.Identity,
                                                 bias=bq2[:, oo:oo + 1], scale=1.0)
                        else:
                            nc.vector.tensor_scalar_add(out=q_sb[:, oo, jts],
                                                        in0=q_ps,
                                                        scalar1=bq2[:, oo:oo + 1])
                # V^T for this tile's four j-chunks: [j on partitions, c free];
                # no bias (folded into bo2), evacuation is a pure copy
                for jc in range(4 * jt, 4 * jt + 4):
                    v_ps = psL.tile([P, ITILE], F32, tag="lps", name=f"vps{jc}")
                    nc.tensor.matmul(v_ps[:, 0:C],
                                     lhsT=x8_sb[:, :, jc * P:(jc + 1) * P],
                                     rhs=wv8[:, :, :],
                                     perf_mode=DR, start=True, stop=True)
                    if jc % 2 == 0:
                        nc.scalar.activation(out=v_sb[:, jc], in_=v_ps[:, 0:C],
                                             func=AF.Copy)
                    else:
                        nc.vector.tensor_copy(out=v_sb[:, jc], in_=v_ps[:, 0:C])

            # residual input, only needed from the first attention tail on
            xres_sb = big.tile([P, 2, HALF], F32, tag="xres")
            nc.sync.dma_start(out=xres_sb, in_=xres_d[:].rearrange("(o p) i -> p o i", p=P))

            # ---- attention (fp8 DoubleRow, software-pipelined in chunk pairs) ----
            NPR = NJC // 2  # 16 key-chunk pairs

            # Schraudolph fast-exp constants (DVE path): for scores s,
            # exp(s*SCALE) ~= bitcast_f32(int32(s*SA + SB)); its ~3% sawtooth
            # error is below the fp8 quantization noise already on P.
            SA = SCALE * 1.4426950408889634 * (1 << 23)
            SB = 127.0 * (1 << 23) - 366000.0 + 0.5
            I32 = mybir.dt.int32

            def emit_s_exp(it, pr, use_dve):
                """Scores for chunk pair (2pr, 2pr+1) and P = exp(S*scale).

                Each DoubleRow matmul contracts the full C=256 via the fp8
                k-interleave; one exp op covers both chunks (2 PSUM banks).
                exp runs on ACT (exact) or DVE (Schraudolph) to split the
                transcendental load across both engines.
                """
                isl = slice(it * ITILE, (it + 1) * ITILE)
                s_ps = psS.tile([P, 2, ITILE], F32, tag="ps")
                for par in range(2):
                    jc = 2 * pr + par
                    jsl = slice(jc * P, (jc + 1) * P)
                    nc.tensor.matmul(s_ps[:, par, :], lhsT=k_sb[:, :, jsl],
                                     rhs=q_sb[:, :, isl],
                                     perf_mode=DR, start=True, stop=True)
                p2 = pp.tile([P, 2, ITILE], FP8, tag="p")
                if use_dve:
                    ti = tip.tile([P, 2, ITILE], I32, tag="ti")
                    nc.vector.tensor_scalar(out=ti, in0=s_ps, scalar1=SA,
                                            scalar2=SB, op0=OP.mult, op1=OP.add)
                    nc.vector.tensor_copy(out=p2, in_=ti.bitcast(F32))
                else:
                    nc.scalar.activation(out=p2, in_=s_ps, func=AF.Exp, scale=SCALE)
                return p2

            def emit_pv(pr, p2, o_ps0, o_ps1, l_ps):
                st, sp = (pr == 0), (pr == NPR - 1)
                nc.tensor.matmul(o_ps0, lhsT=v_sb[:, 2 * pr:2 * pr + 2, 0:P],
                                 rhs=p2, perf_mode=DR, start=st, stop=sp)
                nc.tensor.matmul(o_ps1, lhsT=v_sb[:, 2 * pr:2 * pr + 2, P:C],
                                 rhs=p2, perf_mode=DR, start=st, stop=sp)
                nc.tensor.matmul(l_ps, lhsT=ones8, rhs=p2,
                                 perf_mode=DR, start=st, stop=sp)

            def make_tail_a(it, o_ps0, o_ps1, l_ps):
                def tail_a():
                    recip = rp.tile([P, ITILE], F32, tag="recip")
                    nc.vector.reciprocal_approx_fast(out=recip, in_=l_ps)
                    o_sb = op_pool.tile([P, 2, ITILE], BF16, tag="osb")
                    nc.vector.tensor_tensor(out=o_sb[:, 0], in0=o_ps0,
                                            in1=recip, op=OP.mult)
                    nc.vector.tensor_tensor(out=o_sb[:, 1], in0=o_ps1,
                                            in1=recip, op=OP.mult)
                    return o_sb
                return tail_a

            def make_tail_b(it, o_sb):
                def tail_b():
                    isl = slice(it * ITILE, (it + 1) * ITILE)
                    # output projection + bias + residual
                    for oo in range(2):
                        u_ps = ps1()
                        for ci in range(2):
                            nc.tensor.matmul(
                                u_ps, lhsT=wo_sb[:, ci, oo * P:(oo + 1) * P],
                                rhs=o_sb[:, ci],
                                start=(ci == 0), stop=(ci == 1))
                        res = resp.tile([P, ITILE], F32, tag="res")
                        nc.vector.scalar_tensor_tensor(
                            out=res, in0=u_ps, scalar=bo2[:, oo:oo + 1],
                            in1=xres_sb[:, oo, isl], op0=OP.add, op1=OP.add)
                        nc.sync.dma_start(
                            out=out_d[:].rearrange("(o p) i -> p o i", p=P)[:, oo, isl],
                            in_=res)
                return tail_b

            pending_a = pending_b = None
            for it in range(NIT):
                o_ps0 = psO.tile([P, ITILE], F32, tag="o0")
                o_ps1 = psO.tile([P, ITILE], F32, tag="o1")
                l_ps = psL.tile([P, ITILE], F32, tag="lps")
                pq = []
                for pr in range(NPR):
                    pq.append(emit_s_exp(it, pr, use_dve=False))
                    if pr >= 2:
                        emit_pv(pr - 2, pq[pr - 2], o_ps0, o_ps1, l_ps)
                    if pending_a is not None and pr == 3:
                        o_sb_prev = pending_a()
                        pending_b = make_tail_b(it - 1, o_sb_prev)
                        pending_a = None
                    if pending_b is not None and pr == 6:
                        pending_b()
                        pending_b = None
                for pr in (NPR - 2, NPR - 1):
                    emit_pv(pr, pq[pr], o_ps0, o_ps1, l_ps)
                pending_a = make_tail_a(it, o_ps0, o_ps1, l_ps)
            o_sb_last = pending_a()
            make_tail_b(NIT - 1, o_sb_last)()

    nc.compile()
    return nc


def _get_prog():
    global _PROG
    if _PROG is None:
        _PROG = _build()
    return _PROG


def kernel(x, gn_w, gn_b, wq, bq, wk, bk, wv, bv, wo, bo):
    global _LAST_RESULTS
    import ml_dtypes
    from concourse.bass_utils import run_bass_kernel_spmd

    nc = _get_prog()

    f32 = lambda a: np.ascontiguousarray(np.asarray(a), dtype=np.float32)
    bf16 = lambda a: np.ascontiguousarray(np.asarray(a, dtype=np.float32).astype(ml_dtypes.bfloat16))
    fp8 = lambda a: np.ascontiguousarray(np.asarray(a, dtype=np.float32).astype(ml_dtypes.float8_e4m3fn))
    x = f32(x).reshape(B, C, N)
    shared = {
        "wqT": bf16(np.asarray(wq).T), "wkT": bf16(np.asarray(wk).T),
        "wvT": bf16(np.asarray(wv).T), "woT": bf16(np.asarray(wo).T),
        "bq": f32(bq), "bk": f32(bk), "bv": f32(bv), "bo": f32(bo),
        "gnw": f32(gn_w), "gnb": f32(gn_b),
    }
    a8 = np.zeros((C, GROUPS), np.float32)
    a8[np.arange(C), np.arange(C) // GSIZE] = 1.0 / GSIZE
    shared["a8"] = a8
    e8 = np.zeros((P, C), np.float32)
    e8[np.arange(C) // GSIZE, np.arange(C)] = 1.0
    shared["e8"] = e8

    in_maps = []
    for core in range(NCORES):
        b, h = core // 2, core % 2
        xb = x[b]
        if h == 0:
            xc = xb
        else:
            xc = np.ascontiguousarray(np.concatenate([xb[:, HALF:], xb[:, :HALF]], axis=1))
        m = dict(shared)
        m["xbf"] = bf16(xc)
        m["x8"] = fp8(xc)
        m["xres"] = np.ascontiguousarray(xb[:, h * HALF:(h + 1) * HALF])
        in_maps.append(m)

    _LAST_RESULTS = run_bass_kernel_spmd(nc, in_maps, list(range(NCORES)),
                                         trace=_TRACE)
    out = np.empty((B, C, N), np.float32)
    for core in range(NCORES):
        b, h = core // 2, core % 2
        out[b, :, h * HALF:(h + 1) * HALF] = _LAST_RESULTS.results[core]["out"]
    return out.reshape(B, C, 16, 16, 16)
